# revision 11
# baseline (speedup 1.0000x reference)
"""GAT+JumpingKnowledge GNN kernel for 8 Trainium2 NeuronCores.

Sharding: nodes are partitioned across 8 cores by dst ownership (6250/core).
Each core, per layer:
  - projects its own nodes' features h = x @ [W | W@a_src | W@a_dst]
    (augmented weight precomputed on host, fp16)
  - writes them as packed 256B table rows [64 x fp16 h | f32 alpha_src | f32
    alpha_dst | pad]
  - AllGathers the table (full 50176-row table on every core)
  - gathers, per dst-node "slot grid" (nodes on partitions, incoming-edge
    rounds on the free dim), the src rows of its edges via dma_gather
    (int16 indices -> the table is addressed as two 25088-row halves)
  - edge phase batched over multi-block GROUPS with uniform round counts:
    one DVE instruction per softmax step per group (instruction count, not
    element count, dominates on DVE due to SBUF port arbitration with the
    SWDGE descriptor generator)
Final JK-max + output projection happen on the owned nodes; the host
reassembles, adds the output bias and un-permutes the full [50000, 40]
output.
"""

import numpy as np

# --- problem constants (hardcoded per harness contract) ---
N = 50000
E = 1600000
F_IN = 128
H = 64
L = 3
OUT = 40
NEG_SLOPE = 0.2
NC = 8
NPC_REAL = N // NC          # 6250 real nodes per core
BLOCKS = 49                 # ceil(6250/128)
NPC = BLOCKS * 128          # 6272 padded nodes per core
TAB_ROWS = NC * NPC         # 50176
TAB_HALF = TAB_ROWS // 2    # 25088 (= rows of cores 0..3)
DUMMY_LOCAL = NPC_REAL      # local row 6250 is a pad row on every core
ALPHA_NEG = -1.0e30
GRID_BYTES_BUDGET = 32 * 1024   # per-partition grid bytes per group (A+B)
MAXB = 8                        # max blocks per group


# ---------------------------------------------------------------------------
# Host-side graph preprocessing
# ---------------------------------------------------------------------------

def _fill_grid(Rn, slot_p, rows_vals, dummy):
    """Grid [Rn, 128] in i=r*128+p order; node p's edges fill rounds 0..k-1."""
    grid = np.full((int(Rn), 128), dummy, np.int64)
    o = np.argsort(slot_p, kind="stable")
    ps = slot_p[o]
    rv = rows_vals[o]
    first = np.searchsorted(ps, np.arange(128), side="left")
    ranks = np.arange(len(ps)) - first[ps]
    grid[ranks, ps] = rv
    return grid.reshape(-1)


def _preprocess(edge_index):
    src = np.concatenate([edge_index[0], np.arange(N, dtype=np.int64)]).astype(np.int64)
    dst = np.concatenate([edge_index[1], np.arange(N, dtype=np.int64)]).astype(np.int64)
    is_lo = (src // NPC_REAL) < (NC // 2)   # table half A iff src owned by cores 0-3

    perms = []
    inv_perms = np.zeros((NC, NPC_REAL), np.int64)
    edges_by_core = []
    for c in range(NC):
        lo, hi = c * NPC_REAL, (c + 1) * NPC_REAL
        m = (dst >= lo) & (dst < hi)
        s_c = src[m]
        d_c = dst[m] - lo
        k_lo = np.bincount(d_c[is_lo[m]], minlength=NPC_REAL)
        k_hi = np.bincount(d_c[~is_lo[m]], minlength=NPC_REAL)
        order = np.lexsort((-(k_lo + k_hi), -np.maximum(k_lo, k_hi)))
        perm = np.full(NPC, -1, np.int64)
        perm[:NPC_REAL] = order
        inv_perms[c, order] = np.arange(NPC_REAL)
        perms.append(perm)
        edges_by_core.append((s_c, d_c, k_lo, k_hi))

    def table_row(gids):
        c = gids // NPC_REAL
        return c * NPC + inv_perms[c, gids - c * NPC_REAL]

    # shared (cross-core max) per-block round counts
    RL = np.zeros(BLOCKS, np.int64)
    RH = np.zeros(BLOCKS, np.int64)
    for c in range(NC):
        _, _, k_lo, k_hi = edges_by_core[c]
        perm = perms[c]
        for bidx in range(BLOCKS):
            nodes = perm[bidx * 128:(bidx + 1) * 128]
            nodes = nodes[nodes >= 0]
            if len(nodes):
                RL[bidx] = max(RL[bidx], k_lo[nodes].max())
                RH[bidx] = max(RH[bidx], k_hi[nodes].max())

    # grouping: greedy, uniform rounds per group, grid bytes <= budget
    groups = []  # (start_block, nblocks, RLg, RHg)
    t = 0
    while t < BLOCKS:
        nb = 1
        RLg, RHg = int(RL[t]), int(RH[t])
        while t + nb < BLOCKS and nb < MAXB:
            nRL = max(RLg, int(RL[t + nb]))
            nRH = max(RHg, int(RH[t + nb]))
            if 128 * (nb + 1) * (nRL + nRH) * 2 > GRID_BYTES_BUDGET:
                break
            RLg, RHg = nRL, nRH
            nb += 1
        groups.append((t, nb, RLg, RHg))
        t += nb

    idx_a_cores, idx_b_cores = [], []
    for c in range(NC):
        s_c, d_c, _, _ = edges_by_core[c]
        slot_of = inv_perms[c, d_c]
        rows = table_row(s_c)
        lo_m = rows < TAB_HALF
        la, lb = [], []
        for (t0, nbk, RLg, RHg) in groups:
            for bidx in range(t0, t0 + nbk):
                base = bidx * 128
                in_blk = (slot_of >= base) & (slot_of < base + 128)
                sel = in_blk & lo_m
                la.append(_fill_grid(RLg, slot_of[sel] - base, rows[sel],
                                     DUMMY_LOCAL))
                sel = in_blk & ~lo_m
                lb.append(_fill_grid(RHg, slot_of[sel] - base,
                                     rows[sel] - TAB_HALF, DUMMY_LOCAL))
        idx_a_cores.append(np.concatenate(la).astype(np.int16))
        idx_b_cores.append(np.concatenate(lb).astype(np.int16))

    return perms, idx_a_cores, idx_b_cores, groups


def _pad_neg_col():
    col = np.zeros((128, 1), np.float32)
    col[NPC_REAL - (BLOCKS - 1) * 128:] = ALPHA_NEG
    return col


def _wrap_idx(flat):
    """[num] -> [128, num//16] wrapped (i%16, i//16), replicated to 128 parts."""
    num = len(flat)
    assert num % 16 == 0
    w = flat.reshape(num // 16, 16).T
    return np.ascontiguousarray(np.tile(w, (8, 1))).astype(np.int16)


# ---------------------------------------------------------------------------
# Device kernel builder
# ---------------------------------------------------------------------------

def _build(nc, groups, n_idx_a, n_idx_b):
    import contextlib

    import concourse.mybir as mybir
    import concourse.tile as tile
    from concourse import library_config
    from concourse.masks import make_identity

    f32 = mybir.dt.float32
    f16 = mybir.dt.float16
    AF = mybir.ActivationFunctionType
    ALU = mybir.AluOpType

    # --- I/O ---
    x_in = nc.dram_tensor("x_own", [NPC, F_IN], f16, kind="ExternalInput").ap()
    waug_in = nc.dram_tensor("waug", [L, F_IN, H + 2], f16, kind="ExternalInput").ap()
    bias_in = nc.dram_tensor("bias", [L, H], f32, kind="ExternalInput").ap()
    wout_in = nc.dram_tensor("wout", [H, OUT], f16, kind="ExternalInput").ap()
    idxa_in = nc.dram_tensor("idx_a", [128, n_idx_a // 16], mybir.dt.int16,
                             kind="ExternalInput").ap()
    idxb_in = nc.dram_tensor("idx_b", [128, n_idx_b // 16], mybir.dt.int16,
                             kind="ExternalInput").ap()
    padneg_in = nc.dram_tensor("pad_neg", [128, 1], f32, kind="ExternalInput").ap()
    out_t = nc.dram_tensor("y", [NPC, OUT], f32, kind="ExternalOutput").ap()

    # --- internal DRAM ---
    tab_own = nc.dram_tensor("tab_own", [NPC, H], f32, kind="Internal").ap()
    tab_full = nc.dram_tensor("tab_full", [TAB_ROWS, H], f32, kind="Internal",
                              addr_space="Shared").ap()

    NBR_MAX = max(nbk * (RLg + RHg) for _, nbk, RLg, RHg in groups)
    NB_MAX = max(nbk for _, nbk, _, _ in groups)

    with tile.TileContext(nc) as tc:
        nc.gpsimd.load_library(library_config.mlp)

        with contextlib.ExitStack() as ctx:
            const = ctx.enter_context(tc.tile_pool(name="const", bufs=1))
            psum = ctx.enter_context(tc.tile_pool(name="psum", bufs=2, space="PSUM"))
            sb_pool = ctx.enter_context(tc.tile_pool(name="grids", bufs=4))
            work = ctx.enter_context(tc.tile_pool(name="work", bufs=2))
            small = ctx.enter_context(tc.tile_pool(name="small", bufs=3))

            ident = const.tile([128, 128], f16, tag="ident")
            make_identity(nc, ident[:])
            ones_row = const.tile([1, 128], f32, tag="ones")
            nc.vector.memset(ones_row[:], 1.0)
            idxa_sb = const.tile([128, n_idx_a // 16], mybir.dt.int16, tag="idxa")
            nc.sync.dma_start(idxa_sb[:], idxa_in[:])
            idxb_sb = const.tile([128, n_idx_b // 16], mybir.dt.int16, tag="idxb")
            nc.sync.dma_start(idxb_sb[:], idxb_in[:])
            # x_buf holds layer input as fp16: layer 0 [128, t, F_IN];
            # layers >=1 reuse the first BLOCKS*H columns.
            x_buf = const.tile([128, BLOCKS * F_IN], f16, tag="xbuf")
            nc.sync.dma_start(
                x_buf[:].rearrange("p (t f) -> p t f", t=BLOCKS),
                x_in.rearrange("(t p) f -> p t f", p=128),
            )
            jk_buf = const.tile([128, BLOCKS * H], f16, tag="jkbuf")
            alphad = const.tile([128, BLOCKS], f32, tag="alphad")
            pad_neg = const.tile([128, 1], f32, tag="padneg")
            nc.sync.dma_start(pad_neg[:], padneg_in[:])

            self_q = [0]
            for layer in range(L):
                F = F_IN if layer == 0 else H

                waug = small.tile([128, H + 2], f16, tag="waug")
                nc.sync.dma_start(waug[:F, :], waug_in[layer, :F, :])

                # bias row -> [128, H] broadcast tile
                b_row = small.tile([1, H], f32, tag="brow")
                nc.sync.dma_start(b_row[:], bias_in[layer, None, :])
                bt_ps = psum.tile([128, H], f32, tag="ps_m")
                nc.tensor.matmul(bt_ps[:], ones_row[:], b_row[:],
                                 start=True, stop=True)
                b_tile = small.tile([128, H], f32, tag="btile")
                nc.scalar.copy(b_tile[:], bt_ps[:])

                # project own nodes, pack + store table rows (no DVE here)
                for t in range(BLOCKS):
                    xt = x_buf[:, t * F:(t + 1) * F]
                    xT_ps = psum.tile([F, 128], f16, tag="ps_t")
                    nc.tensor.transpose(xT_ps[:], xt, ident[:])
                    xT_sb = work.tile([F, 128], f16, tag="xTsb")
                    nc.scalar.copy(xT_sb[:], xT_ps[:])
                    h_ps = psum.tile([128, H + 2], f32, tag="ps_m")
                    nc.tensor.matmul(h_ps[:], xT_sb[:], waug[:F, :],
                                     start=True, stop=True)
                    row = work.tile([128, H], f32, tag="row")
                    row16 = row[:].bitcast(f16)
                    nc.scalar.copy(row16[:, 0:H], h_ps[:, 0:H])
                    if t == BLOCKS - 1:
                        # pad rows (incl. the dummy target row): alpha_src -> -1e30
                        nc.scalar.activation(row[:, 32:33], h_ps[:, H:H + 1],
                                             AF.Identity, bias=pad_neg[:, 0:1])
                        nc.scalar.copy(row[:, 33:34], h_ps[:, H + 1:H + 2])
                    else:
                        nc.scalar.copy(row[:, 32:34], h_ps[:, H:H + 2])
                    nc.scalar.copy(alphad[:, t:t + 1], h_ps[:, H + 1:H + 2])
                    nc.sync.dma_start(tab_own[t * 128:(t + 1) * 128, :], row[:])

                nc.gpsimd.collective_compute(
                    "AllGather",
                    ALU.bypass,
                    replica_groups=[list(range(NC))],
                    ins=[tab_own.opt()],
                    outs=[tab_full.opt()],
                )

                # edge processing, batched per group (single unified grid:
                # block b occupies rounds [b*Rg, (b+1)*Rg), A-half first)
                off_a = 0
                off_b = 0
                for (t0, nbk, RLg, RHg) in groups:
                    Rg = RLg + RHg
                    nbr = nbk * Rg
                    grid = sb_pool.tile([128, nbr * H], f32, tag="grid")
                    g3 = grid[:].rearrange("p (r h) -> p r h", h=H)
                    # dma_gather is capped at 1024 indices per call (SWDGE
                    # descriptor ring); per (block, half) region, split and
                    # round-robin the queues.
                    for bi in range(nbk):
                        for RX, roff, off, total_off, isb, base in (
                            (RLg, bi * Rg, off_a + bi * 128 * RLg, 0,
                             idxa_sb, tab_full[0:TAB_HALF, :]),
                            (RHg, bi * Rg + RLg, off_b + bi * 128 * RHg, 0,
                             idxb_sb, tab_full[TAB_HALF:TAB_ROWS, :]),
                        ):
                            n_tot = 128 * RX
                            done = 0
                            while done < n_tot:
                                step = min(1024, n_tot - done)
                                nc.gpsimd.dma_gather(
                                    g3[:, roff + done // 128:
                                       roff + (done + step) // 128, :],
                                    base,
                                    isb[:, (off + done) // 16:
                                        (off + done + step) // 16],
                                    step, step, H,
                                    queue_num=self_q[0] % 4,
                                )
                                self_q[0] += 1
                                done += step
                    off_a += 128 * nbk * RLg
                    off_b += 128 * nbk * RHg

                    # e = alpha_src + alpha_dst per block on ACT (one call per
                    # block: A and B rounds are contiguous in the unified grid)
                    tbuf = work.tile([128, NBR_MAX], f32, tag="tbuf")
                    for bi in range(nbk):
                        nc.scalar.activation(
                            tbuf[:, bi * Rg:(bi + 1) * Rg],
                            g3[:, bi * Rg:(bi + 1) * Rg, 32], AF.Identity,
                            bias=alphad[:, t0 + bi:t0 + bi + 1])
                    # leaky relu over the whole group
                    nc.vector.scalar_tensor_tensor(
                        out=tbuf[:, 0:nbr], in0=tbuf[:, 0:nbr],
                        scalar=NEG_SLOPE, in1=tbuf[:, 0:nbr],
                        op0=ALU.mult, op1=ALU.max)
                    tb3 = tbuf[:, 0:nbr].rearrange("p (b r) -> p b r", r=Rg)
                    m_neg = small.tile([128, NB_MAX], f32, tag="mneg")
                    nc.vector.reduce_max(m_neg[:, 0:nbk], tb3,
                                         axis=mybir.AxisListType.X, negate=True)
                    p16 = work.tile([128, NBR_MAX], f16, tag="ptile")
                    den = small.tile([128, NB_MAX], f32, tag="den")
                    for bi in range(nbk):
                        nc.scalar.activation(
                            p16[:, bi * Rg:(bi + 1) * Rg],
                            tbuf[:, bi * Rg:(bi + 1) * Rg], AF.Exp,
                            bias=m_neg[:, bi:bi + 1],
                            accum_out=den[:, bi:bi + 1])
                    recip = small.tile([128, NB_MAX], f32, tag="recip")
                    nc.vector.reciprocal(recip[:, 0:nbk], den[:, 0:nbk])
                    coef = work.tile([128, NBR_MAX], f16, tag="coef")
                    c3 = coef[:, 0:nbr].rearrange("p (b r) -> p b r", r=Rg)
                    nc.vector.tensor_tensor(
                        out=c3,
                        in0=p16[:, 0:nbr].rearrange("p (b r) -> p b r", r=Rg),
                        in1=recip[:, 0:nbk].unsqueeze(2).to_broadcast(
                            [128, nbk, Rg]),
                        op=ALU.mult)
                    # weighted messages: in-place fp16 multiply (contiguous),
                    # strided-view reduce; one instruction each for the group.
                    numA = work.tile([128, NB_MAX * H], f32, tag="numA")
                    hU = (grid[:].bitcast(f16)
                          .rearrange("p (b r h) -> p b r h", b=nbk, h=2 * H)
                          [:, :, :, 0:H])
                    nc.vector.tensor_tensor(
                        out=hU, in0=hU,
                        in1=c3.unsqueeze(3).to_broadcast([128, nbk, Rg, H]),
                        op=ALU.mult)
                    nA3 = (numA[:, 0:nbk * H]
                           .rearrange("p (b h) -> p b h", b=nbk))
                    nc.vector.reduce_sum(nA3, hU.transpose([0, 1, 3, 2]),
                                         axis=mybir.AxisListType.X)
                    nbh = nbk * H
                    nc.vector.tensor_tensor(
                        out=nA3, in0=nA3,
                        in1=b_tile[:].unsqueeze(1).to_broadcast([128, nbk, H]),
                        op=ALU.add)
                    if layer < L - 1:
                        xn = x_buf[:, t0 * H:(t0 + nbk) * H]
                    else:
                        xn = work.tile([128, NB_MAX * H], f16, tag="xnlast",
                                       name="xnlast")[:, 0:nbh]
                    nc.vector.tensor_scalar_max(xn, numA[:, 0:nbh], 0.0)
                    jk = jk_buf[:, t0 * H:(t0 + nbk) * H]
                    if layer == 0:
                        nc.vector.tensor_copy(jk, xn)
                    else:
                        nc.vector.tensor_tensor(out=jk, in0=jk, in1=xn,
                                                op=ALU.max)

            # final projection: y = jk @ Wout  (bout added on host)
            wout_sb = const.tile([H, OUT], f16, tag="wout")
            nc.sync.dma_start(wout_sb[:], wout_in[:])
            for t in range(BLOCKS):
                jt = jk_buf[:, t * H:(t + 1) * H]
                jT_ps = psum.tile([H, 128], f16, tag="ps_t")
                nc.tensor.transpose(jT_ps[:], jt, ident[:])
                jT_sb = work.tile([H, 128], f16, tag="jTsb")
                nc.scalar.copy(jT_sb[:], jT_ps[:])
                y_ps = psum.tile([128, OUT], f32, tag="ps_m")
                nc.tensor.matmul(y_ps[:], jT_sb[:], wout_sb[:], start=True, stop=True)
                y_sb = work.tile([128, OUT], f32, tag="ysb")
                nc.scalar.copy(y_sb[:], y_ps[:])
                nc.sync.dma_start(out_t[t * 128:(t + 1) * 128, :], y_sb[:])

    return nc


# ---------------------------------------------------------------------------
# Entry point
# ---------------------------------------------------------------------------

def kernel(x, edge_index, W1, W23, a_src, a_dst, b, Wout, bout):
    import concourse.bacc as bacc
    from concourse import bass_utils

    x = np.asarray(x, np.float32)
    edge_index = np.asarray(edge_index)
    perms, idx_a, idx_b, groups = _preprocess(edge_index.astype(np.int64))

    n_idx_a = len(idx_a[0])
    n_idx_b = len(idx_b[0])

    # host-precomputed augmented weights [W | W@a_src | W@a_dst], fp16
    waug = np.zeros((L, F_IN, H + 2), np.float16)
    W1 = np.asarray(W1, np.float32)
    W23 = np.asarray(W23, np.float32)
    a_src = np.asarray(a_src, np.float32)
    a_dst = np.asarray(a_dst, np.float32)
    for l in range(L):
        W = W1 if l == 0 else W23[l - 1]
        Fl = W.shape[0]
        waug[l, :Fl, 0:H] = W.astype(np.float16)
        waug[l, :Fl, H] = (W @ a_src[l]).astype(np.float16)
        waug[l, :Fl, H + 1] = (W @ a_dst[l]).astype(np.float16)

    nc = bacc.Bacc("TRN2", target_bir_lowering=False, debug=False, num_devices=NC,
                   num_swdge_queues=4)
    _build(nc, groups, n_idx_a, n_idx_b)
    nc.compile()

    in_maps = []
    for c in range(NC):
        perm = perms[c]
        x_own = np.zeros((NPC, F_IN), np.float16)
        valid = np.nonzero(perm >= 0)[0]
        x_own[valid] = x[c * NPC_REAL + perm[valid]].astype(np.float16)
        in_maps.append({
            "x_own": x_own,
            "waug": waug,
            "bias": np.asarray(b, np.float32),
            "wout": np.asarray(Wout, np.float16),
            "idx_a": _wrap_idx(idx_a[c]),
            "idx_b": _wrap_idx(idx_b[c]),
            "pad_neg": _pad_neg_col(),
        })

    res = bass_utils.run_bass_kernel_spmd(nc, in_maps, core_ids=list(range(NC)))
    global _last_results
    _last_results = res
    bout = np.asarray(bout, np.float32).reshape(1, OUT)
    out = np.zeros((N, OUT), np.float32)
    for c in range(NC):
        y = res.results[c]["y"]
        perm = perms[c]
        valid = np.nonzero(perm >= 0)[0]
        out[c * NPC_REAL + perm[valid]] = y[valid] + bout
    return out


# revision 12
# speedup vs baseline: 1.0326x; 1.0326x over previous
"""GAT+JumpingKnowledge GNN kernel for 8 Trainium2 NeuronCores.

Sharding: nodes are partitioned across 8 cores by dst ownership (6250/core).
Each core, per layer:
  - projects its own nodes' features h = x @ [W | W@a_src | W@a_dst]
    (augmented weight precomputed on host, fp16)
  - writes them as packed 256B table rows [64 x fp16 h | f32 alpha_src | f32
    alpha_dst | pad]
  - AllGathers the table (full 50176-row table on every core)
  - gathers, per dst-node "slot grid" (nodes on partitions, incoming-edge
    rounds on the free dim), the src rows of its edges via dma_gather
    (int16 indices -> the table is addressed as two 25088-row halves)
  - edge phase batched over multi-block GROUPS with uniform round counts:
    one DVE instruction per softmax step per group (instruction count, not
    element count, dominates on DVE due to SBUF port arbitration with the
    SWDGE descriptor generator)
Final JK-max + output projection happen on the owned nodes; the host
reassembles, adds the output bias and un-permutes the full [50000, 40]
output.
"""

import numpy as np

# --- problem constants (hardcoded per harness contract) ---
N = 50000
E = 1600000
F_IN = 128
H = 64
L = 3
OUT = 40
NEG_SLOPE = 0.2
NC = 8
NPC_REAL = N // NC          # 6250 real nodes per core
BLOCKS = 49                 # ceil(6250/128)
NPC = BLOCKS * 128          # 6272 padded nodes per core
TAB_ROWS = NC * NPC         # 50176
TAB_HALF = TAB_ROWS // 2    # 25088 (= rows of cores 0..3)
DUMMY_LOCAL = NPC_REAL      # local row 6250 is a pad row on every core
ALPHA_NEG = -1.0e30
GRID_BYTES_BUDGET = 20 * 1024   # per-partition grid bytes per group (A+B)
MAXB = 6                        # max blocks per group


# ---------------------------------------------------------------------------
# Host-side graph preprocessing
# ---------------------------------------------------------------------------

def _fill_grid(Rn, slot_p, rows_vals, dummy):
    """Grid [Rn, 128] in i=r*128+p order; node p's edges fill rounds 0..k-1."""
    grid = np.full((int(Rn), 128), dummy, np.int64)
    o = np.argsort(slot_p, kind="stable")
    ps = slot_p[o]
    rv = rows_vals[o]
    first = np.searchsorted(ps, np.arange(128), side="left")
    ranks = np.arange(len(ps)) - first[ps]
    grid[ranks, ps] = rv
    return grid.reshape(-1)


def _preprocess(edge_index):
    src = np.concatenate([edge_index[0], np.arange(N, dtype=np.int64)]).astype(np.int64)
    dst = np.concatenate([edge_index[1], np.arange(N, dtype=np.int64)]).astype(np.int64)
    is_lo = (src // NPC_REAL) < (NC // 2)   # table half A iff src owned by cores 0-3

    perms = []
    inv_perms = np.zeros((NC, NPC_REAL), np.int64)
    edges_by_core = []
    for c in range(NC):
        lo, hi = c * NPC_REAL, (c + 1) * NPC_REAL
        m = (dst >= lo) & (dst < hi)
        s_c = src[m]
        d_c = dst[m] - lo
        k_lo = np.bincount(d_c[is_lo[m]], minlength=NPC_REAL)
        k_hi = np.bincount(d_c[~is_lo[m]], minlength=NPC_REAL)
        order = np.lexsort((-(k_lo + k_hi), -np.maximum(k_lo, k_hi)))
        perm = np.full(NPC, -1, np.int64)
        perm[:NPC_REAL] = order
        inv_perms[c, order] = np.arange(NPC_REAL)
        perms.append(perm)
        edges_by_core.append((s_c, d_c, k_lo, k_hi))

    def table_row(gids):
        c = gids // NPC_REAL
        return c * NPC + inv_perms[c, gids - c * NPC_REAL]

    # shared (cross-core max) per-block round counts
    RL = np.zeros(BLOCKS, np.int64)
    RH = np.zeros(BLOCKS, np.int64)
    for c in range(NC):
        _, _, k_lo, k_hi = edges_by_core[c]
        perm = perms[c]
        for bidx in range(BLOCKS):
            nodes = perm[bidx * 128:(bidx + 1) * 128]
            nodes = nodes[nodes >= 0]
            if len(nodes):
                RL[bidx] = max(RL[bidx], k_lo[nodes].max())
                RH[bidx] = max(RH[bidx], k_hi[nodes].max())

    # grouping: greedy, uniform rounds per group, grid bytes <= budget
    groups = []  # (start_block, nblocks, RLg, RHg)
    t = 0
    while t < BLOCKS:
        nb = 1
        RLg, RHg = int(RL[t]), int(RH[t])
        while t + nb < BLOCKS and nb < MAXB:
            nRL = max(RLg, int(RL[t + nb]))
            nRH = max(RHg, int(RH[t + nb]))
            if 128 * (nb + 1) * (nRL + nRH) * 2 > GRID_BYTES_BUDGET:
                break
            RLg, RHg = nRL, nRH
            nb += 1
        groups.append((t, nb, RLg, RHg))
        t += nb

    idx_a_cores, idx_b_cores = [], []
    for c in range(NC):
        s_c, d_c, _, _ = edges_by_core[c]
        slot_of = inv_perms[c, d_c]
        rows = table_row(s_c)
        lo_m = rows < TAB_HALF
        la, lb = [], []
        for (t0, nbk, RLg, RHg) in groups:
            for bidx in range(t0, t0 + nbk):
                base = bidx * 128
                in_blk = (slot_of >= base) & (slot_of < base + 128)
                sel = in_blk & lo_m
                la.append(_fill_grid(RLg, slot_of[sel] - base, rows[sel],
                                     DUMMY_LOCAL))
                sel = in_blk & ~lo_m
                lb.append(_fill_grid(RHg, slot_of[sel] - base,
                                     rows[sel] - TAB_HALF, DUMMY_LOCAL))
        idx_a_cores.append(np.concatenate(la).astype(np.int16))
        idx_b_cores.append(np.concatenate(lb).astype(np.int16))

    return perms, idx_a_cores, idx_b_cores, groups


def _pad_neg_col():
    col = np.zeros((128, 1), np.float32)
    col[NPC_REAL - (BLOCKS - 1) * 128:] = ALPHA_NEG
    return col


def _wrap_idx(flat):
    """[num] -> [128, num//16] wrapped (i%16, i//16), replicated to 128 parts."""
    num = len(flat)
    assert num % 16 == 0
    w = flat.reshape(num // 16, 16).T
    return np.ascontiguousarray(np.tile(w, (8, 1))).astype(np.int16)


# ---------------------------------------------------------------------------
# Device kernel builder
# ---------------------------------------------------------------------------

def _build(nc, groups, n_idx_a, n_idx_b):
    import contextlib

    import concourse.mybir as mybir
    import concourse.tile as tile
    from concourse import library_config
    from concourse.masks import make_identity

    f32 = mybir.dt.float32
    f16 = mybir.dt.float16
    AF = mybir.ActivationFunctionType
    ALU = mybir.AluOpType

    # --- I/O ---
    x_in = nc.dram_tensor("x_own", [NPC, F_IN], f16, kind="ExternalInput").ap()
    waug_in = nc.dram_tensor("waug", [L, F_IN, H + 2], f16, kind="ExternalInput").ap()
    bias_in = nc.dram_tensor("bias", [L, H], f32, kind="ExternalInput").ap()
    wout_in = nc.dram_tensor("wout", [H, OUT], f16, kind="ExternalInput").ap()
    idxa_in = nc.dram_tensor("idx_a", [128, n_idx_a // 16], mybir.dt.int16,
                             kind="ExternalInput").ap()
    idxb_in = nc.dram_tensor("idx_b", [128, n_idx_b // 16], mybir.dt.int16,
                             kind="ExternalInput").ap()
    padneg_in = nc.dram_tensor("pad_neg", [128, 1], f32, kind="ExternalInput").ap()
    out_t = nc.dram_tensor("y", [NPC, OUT], f32, kind="ExternalOutput").ap()

    # --- internal DRAM ---
    tab_own = nc.dram_tensor("tab_own", [NPC, H], f32, kind="Internal").ap()
    tab_full = nc.dram_tensor("tab_full", [TAB_ROWS, H], f32, kind="Internal",
                              addr_space="Shared").ap()

    NBR_MAX = max(nbk * (RLg + RHg) for _, nbk, RLg, RHg in groups)
    NB_MAX = max(nbk for _, nbk, _, _ in groups)

    with tile.TileContext(nc) as tc:
        nc.gpsimd.load_library(library_config.mlp)

        with contextlib.ExitStack() as ctx:
            const = ctx.enter_context(tc.tile_pool(name="const", bufs=1))
            psum = ctx.enter_context(tc.tile_pool(name="psum", bufs=2, space="PSUM"))
            sb_pool = ctx.enter_context(tc.tile_pool(name="grids", bufs=4))
            work = ctx.enter_context(tc.tile_pool(name="work", bufs=2))
            small = ctx.enter_context(tc.tile_pool(name="small", bufs=3))

            ident = const.tile([128, 128], f16, tag="ident")
            make_identity(nc, ident[:])
            ones_row = const.tile([1, 128], f32, tag="ones")
            nc.vector.memset(ones_row[:], 1.0)
            idxa_sb = const.tile([128, n_idx_a // 16], mybir.dt.int16, tag="idxa")
            nc.sync.dma_start(idxa_sb[:], idxa_in[:])
            idxb_sb = const.tile([128, n_idx_b // 16], mybir.dt.int16, tag="idxb")
            nc.sync.dma_start(idxb_sb[:], idxb_in[:])
            # x_buf holds layer input as fp16: layer 0 [128, t, F_IN];
            # layers >=1 reuse the first BLOCKS*H columns.
            x_buf = const.tile([128, BLOCKS * F_IN], f16, tag="xbuf")
            nc.sync.dma_start(
                x_buf[:].rearrange("p (t f) -> p t f", t=BLOCKS),
                x_in.rearrange("(t p) f -> p t f", p=128),
            )
            jk_buf = const.tile([128, BLOCKS * H], f16, tag="jkbuf")
            alphad = const.tile([128, BLOCKS], f32, tag="alphad")
            pad_neg = const.tile([128, 1], f32, tag="padneg")
            nc.sync.dma_start(pad_neg[:], padneg_in[:])

            self_q = [0]
            for layer in range(L):
                F = F_IN if layer == 0 else H

                waug = small.tile([128, H + 2], f16, tag="waug")
                nc.sync.dma_start(waug[:F, :], waug_in[layer, :F, :])

                # bias row -> [128, H] broadcast tile
                b_row = small.tile([1, H], f32, tag="brow")
                nc.sync.dma_start(b_row[:], bias_in[layer, None, :])
                bt_ps = psum.tile([128, H], f32, tag="ps_m")
                nc.tensor.matmul(bt_ps[:], ones_row[:], b_row[:],
                                 start=True, stop=True)
                b_tile = small.tile([128, H], f32, tag="btile")
                nc.scalar.copy(b_tile[:], bt_ps[:])

                # project own nodes, pack + store table rows (no DVE here)
                for t in range(BLOCKS):
                    xt = x_buf[:, t * F:(t + 1) * F]
                    xT_ps = psum.tile([F, 128], f16, tag="ps_t")
                    nc.tensor.transpose(xT_ps[:], xt, ident[:])
                    xT_sb = work.tile([F, 128], f16, tag="xTsb")
                    nc.scalar.copy(xT_sb[:], xT_ps[:])
                    h_ps = psum.tile([128, H + 2], f32, tag="ps_m")
                    nc.tensor.matmul(h_ps[:], xT_sb[:], waug[:F, :],
                                     start=True, stop=True)
                    row = work.tile([128, H], f32, tag="row")
                    row16 = row[:].bitcast(f16)
                    nc.scalar.copy(row16[:, 0:H], h_ps[:, 0:H])
                    if t == BLOCKS - 1:
                        # pad rows (incl. the dummy target row): alpha_src -> -1e30
                        nc.scalar.activation(row[:, 32:33], h_ps[:, H:H + 1],
                                             AF.Identity, bias=pad_neg[:, 0:1])
                        nc.scalar.copy(row[:, 33:34], h_ps[:, H + 1:H + 2])
                    else:
                        nc.scalar.copy(row[:, 32:34], h_ps[:, H:H + 2])
                    nc.scalar.copy(alphad[:, t:t + 1], h_ps[:, H + 1:H + 2])
                    nc.sync.dma_start(tab_own[t * 128:(t + 1) * 128, :], row[:])

                nc.gpsimd.collective_compute(
                    "AllGather",
                    ALU.bypass,
                    replica_groups=[list(range(NC))],
                    ins=[tab_own.opt()],
                    outs=[tab_full.opt()],
                )

                # edge processing, batched per group
                off_a = 0
                off_b = 0
                for (t0, nbk, RLg, RHg) in groups:
                    Rg = RLg + RHg
                    nbr = nbk * Rg
                    na = 128 * nbk * RLg
                    nbt = 128 * nbk * RHg
                    ga = sb_pool.tile([128, max(na // 128, 1) * H], f32, tag="gridA")
                    gb = sb_pool.tile([128, max(nbt // 128, 1) * H], f32, tag="gridB")
                    # dma_gather is capped at 1024 indices per call (SWDGE
                    # descriptor ring); split and round-robin the queues.
                    for grid, n_tot, off, isb, base in (
                        (ga, na, off_a, idxa_sb, tab_full[0:TAB_HALF, :]),
                        (gb, nbt, off_b, idxb_sb, tab_full[TAB_HALF:TAB_ROWS, :]),
                    ):
                        done = 0
                        while done < n_tot:
                            step = min(1024, n_tot - done)
                            nc.gpsimd.dma_gather(
                                grid[:].rearrange("p (r h) -> p r h", h=H)
                                [:, done // 128:(done + step) // 128, :],
                                base,
                                isb[:, (off + done) // 16:(off + done + step) // 16],
                                step, step, H,
                                queue_num=self_q[0] % 4,
                            )
                            self_q[0] += 1
                            done += step
                    off_a += na
                    off_b += nbt

                    ga3 = ga[:].rearrange("p (r h) -> p r h", h=H)
                    gb3 = gb[:].rearrange("p (r h) -> p r h", h=H)
                    # e = alpha_src + alpha_dst per (block, half) on ACT
                    tbuf = work.tile([128, NBR_MAX], f32, tag="tbuf")
                    for bi in range(nbk):
                        nc.scalar.activation(
                            tbuf[:, bi * Rg: bi * Rg + RLg],
                            ga3[:, bi * RLg:(bi + 1) * RLg, 32], AF.Identity,
                            bias=alphad[:, t0 + bi:t0 + bi + 1])
                        nc.scalar.activation(
                            tbuf[:, bi * Rg + RLg:(bi + 1) * Rg],
                            gb3[:, bi * RHg:(bi + 1) * RHg, 32], AF.Identity,
                            bias=alphad[:, t0 + bi:t0 + bi + 1])
                    # leaky relu over the whole group
                    nc.vector.scalar_tensor_tensor(
                        out=tbuf[:, 0:nbr], in0=tbuf[:, 0:nbr],
                        scalar=NEG_SLOPE, in1=tbuf[:, 0:nbr],
                        op0=ALU.mult, op1=ALU.max)
                    tb3 = tbuf[:, 0:nbr].rearrange("p (b r) -> p b r", r=Rg)
                    m_neg = small.tile([128, NB_MAX], f32, tag="mneg")
                    nc.vector.reduce_max(m_neg[:, 0:nbk], tb3,
                                         axis=mybir.AxisListType.X, negate=True)
                    p16 = work.tile([128, NBR_MAX], f16, tag="ptile")
                    den = small.tile([128, NB_MAX], f32, tag="den")
                    for bi in range(nbk):
                        nc.scalar.activation(
                            p16[:, bi * Rg:(bi + 1) * Rg],
                            tbuf[:, bi * Rg:(bi + 1) * Rg], AF.Exp,
                            bias=m_neg[:, bi:bi + 1],
                            accum_out=den[:, bi:bi + 1])
                    recip = small.tile([128, NB_MAX], f32, tag="recip")
                    nc.vector.reciprocal(recip[:, 0:nbk], den[:, 0:nbk])
                    coef = work.tile([128, NBR_MAX], f16, tag="coef")
                    c3 = coef[:, 0:nbr].rearrange("p (b r) -> p b r", r=Rg)
                    nc.vector.tensor_tensor(
                        out=c3,
                        in0=p16[:, 0:nbr].rearrange("p (b r) -> p b r", r=Rg),
                        in1=recip[:, 0:nbk].unsqueeze(2).to_broadcast(
                            [128, nbk, Rg]),
                        op=ALU.mult)
                    # weighted messages: in-place fp16 multiply (contiguous),
                    # then strided-view reduce; one instruction per half.
                    numA = work.tile([128, NB_MAX * H], f32, tag="numA")
                    numB = work.tile([128, NB_MAX * H], f32, tag="numB")
                    hA = (ga[:].bitcast(f16)
                          .rearrange("p (b r h) -> p b r h", b=nbk, h=2 * H)
                          [:, :, :, 0:H])
                    nc.vector.tensor_tensor(
                        out=hA, in0=hA,
                        in1=c3[:, :, 0:RLg].unsqueeze(3).to_broadcast(
                            [128, nbk, RLg, H]),
                        op=ALU.mult)
                    nA3 = (numA[:, 0:nbk * H]
                           .rearrange("p (b h) -> p b h", b=nbk))
                    nc.vector.reduce_sum(nA3, hA.transpose([0, 1, 3, 2]),
                                         axis=mybir.AxisListType.X)
                    hB = (gb[:].bitcast(f16)
                          .rearrange("p (b r h) -> p b r h", b=nbk, h=2 * H)
                          [:, :, :, 0:H])
                    nc.vector.tensor_tensor(
                        out=hB, in0=hB,
                        in1=c3[:, :, RLg:Rg].unsqueeze(3).to_broadcast(
                            [128, nbk, RHg, H]),
                        op=ALU.mult)
                    nB3 = (numB[:, 0:nbk * H]
                           .rearrange("p (b h) -> p b h", b=nbk))
                    nc.vector.reduce_sum(nB3, hB.transpose([0, 1, 3, 2]),
                                         axis=mybir.AxisListType.X)
                    nbh = nbk * H
                    nc.vector.tensor_tensor(out=numA[:, 0:nbh], in0=numA[:, 0:nbh],
                                            in1=numB[:, 0:nbh], op=ALU.add)
                    nc.vector.tensor_tensor(
                        out=nA3, in0=nA3,
                        in1=b_tile[:].unsqueeze(1).to_broadcast([128, nbk, H]),
                        op=ALU.add)
                    if layer < L - 1:
                        xn = x_buf[:, t0 * H:(t0 + nbk) * H]
                    else:
                        xn = work.tile([128, NB_MAX * H], f16, tag="xnlast",
                                       name="xnlast")[:, 0:nbh]
                    nc.vector.tensor_scalar_max(xn, numA[:, 0:nbh], 0.0)
                    jk = jk_buf[:, t0 * H:(t0 + nbk) * H]
                    if layer == 0:
                        nc.vector.tensor_copy(jk, xn)
                    else:
                        nc.vector.tensor_tensor(out=jk, in0=jk, in1=xn,
                                                op=ALU.max)

            # final projection: y = jk @ Wout  (bout added on host)
            wout_sb = const.tile([H, OUT], f16, tag="wout")
            nc.sync.dma_start(wout_sb[:], wout_in[:])
            for t in range(BLOCKS):
                jt = jk_buf[:, t * H:(t + 1) * H]
                jT_ps = psum.tile([H, 128], f16, tag="ps_t")
                nc.tensor.transpose(jT_ps[:], jt, ident[:])
                jT_sb = work.tile([H, 128], f16, tag="jTsb")
                nc.scalar.copy(jT_sb[:], jT_ps[:])
                y_ps = psum.tile([128, OUT], f32, tag="ps_m")
                nc.tensor.matmul(y_ps[:], jT_sb[:], wout_sb[:], start=True, stop=True)
                y_sb = work.tile([128, OUT], f32, tag="ysb")
                nc.scalar.copy(y_sb[:], y_ps[:])
                nc.sync.dma_start(out_t[t * 128:(t + 1) * 128, :], y_sb[:])

    return nc


# ---------------------------------------------------------------------------
# Entry point
# ---------------------------------------------------------------------------

def kernel(x, edge_index, W1, W23, a_src, a_dst, b, Wout, bout):
    import concourse.bacc as bacc
    from concourse import bass_utils

    x = np.asarray(x, np.float32)
    edge_index = np.asarray(edge_index)
    perms, idx_a, idx_b, groups = _preprocess(edge_index.astype(np.int64))

    n_idx_a = len(idx_a[0])
    n_idx_b = len(idx_b[0])

    # host-precomputed augmented weights [W | W@a_src | W@a_dst], fp16
    waug = np.zeros((L, F_IN, H + 2), np.float16)
    W1 = np.asarray(W1, np.float32)
    W23 = np.asarray(W23, np.float32)
    a_src = np.asarray(a_src, np.float32)
    a_dst = np.asarray(a_dst, np.float32)
    for l in range(L):
        W = W1 if l == 0 else W23[l - 1]
        Fl = W.shape[0]
        waug[l, :Fl, 0:H] = W.astype(np.float16)
        waug[l, :Fl, H] = (W @ a_src[l]).astype(np.float16)
        waug[l, :Fl, H + 1] = (W @ a_dst[l]).astype(np.float16)

    nc = bacc.Bacc("TRN2", target_bir_lowering=False, debug=False, num_devices=NC,
                   num_swdge_queues=4)
    _build(nc, groups, n_idx_a, n_idx_b)
    nc.compile()

    in_maps = []
    for c in range(NC):
        perm = perms[c]
        x_own = np.zeros((NPC, F_IN), np.float16)
        valid = np.nonzero(perm >= 0)[0]
        x_own[valid] = x[c * NPC_REAL + perm[valid]].astype(np.float16)
        in_maps.append({
            "x_own": x_own,
            "waug": waug,
            "bias": np.asarray(b, np.float32),
            "wout": np.asarray(Wout, np.float16),
            "idx_a": _wrap_idx(idx_a[c]),
            "idx_b": _wrap_idx(idx_b[c]),
            "pad_neg": _pad_neg_col(),
        })

    res = bass_utils.run_bass_kernel_spmd(nc, in_maps, core_ids=list(range(NC)))
    global _last_results
    _last_results = res
    bout = np.asarray(bout, np.float32).reshape(1, OUT)
    out = np.zeros((N, OUT), np.float32)
    for c in range(NC):
        y = res.results[c]["y"]
        perm = perms[c]
        valid = np.nonzero(perm >= 0)[0]
        out[c * NPC_REAL + perm[valid]] = y[valid] + bout
    return out
